# revision 1
# baseline (speedup 1.0000x reference)
"""Trainium2 Bass kernel for nn_CrossModalGNNLayer (M=8192, D=128, DEG=32).

out = leaky_relu(local + global + z)
  local[i]  = sum_{k=1..32} alpha[i,k] * wg[(i+k)%M]   (banded GAT attention)
  global    = softmax(q k^T / sqrt(d)) @ (z Wc^T)       (dense attention)

Sharding: rows split across 8 cores (1024 each).  Each core gets the full z
(two layouts) plus a per-core halo slice; no collectives.  The dense branch is
computed flash-style in a transposed [key, query] layout so the PV matmul needs
no transposes; the score matrix never touches HBM.

Host-side prep is layout-only (transposes / gathers / the 128x128 product
Wq^T Wk); all FLOPs over node data run on device.
"""

import math
import os
import numpy as np
from contextlib import ExitStack

M = 8192
D = 128
DEG = 32
NCORES = 8
ROWS = M // NCORES          # 1024 rows per core
HALO = 1280                 # per-core halo columns of z^T (r0 .. r0+1279)
J = 512                     # query-block size for the dense branch
NQ = M // 128               # 64 key chunks
NB = ROWS // J              # 2 query blocks per core
NT = J // 128               # 4 sub-blocks per query block
NL = ROWS // 128            # 8 local band blocks per core
BAND = 160                  # 128 + DEG columns per band block
LEAK = 0.01                 # jax.nn.leaky_relu default slope
SCALE = 1.0 / math.sqrt(D)

# matmul input dtype: float32 (exact) or float32r (4x faster, slightly relaxed)
MM_DTYPE = os.environ.get("KERNEL_MM_DTYPE", "float32")

_CACHE = {}


def _build_nc():
    import concourse.bass as bass  # noqa: F401
    import concourse.tile as tile
    from concourse import bacc, mybir
    from concourse.masks import make_identity

    f32 = mybir.dt.float32
    mm_dt = getattr(mybir.dt, MM_DTYPE)
    Act = mybir.ActivationFunctionType
    Alu = mybir.AluOpType
    AX = mybir.AxisListType.X

    nc = bacc.Bacc("TRN2", target_bir_lowering=False, debug=False)

    zT = nc.dram_tensor("zT", [D, M], f32, kind="ExternalInput")
    zc = nc.dram_tensor("zc", [128, NQ, D], f32, kind="ExternalInput")
    zTh = nc.dram_tensor("zTh", [D, HALO], f32, kind="ExternalInput")
    zoc = nc.dram_tensor("zoc", [128, NL, D], f32, kind="ExternalInput")
    BT = nc.dram_tensor("BT", [D, D], f32, kind="ExternalInput")
    Wg_t = nc.dram_tensor("Wg_t", [D, D], f32, kind="ExternalInput")
    Wc_t = nc.dram_tensor("Wc_t", [D, D], f32, kind="ExternalInput")
    a_cols = nc.dram_tensor("a_cols", [D, 2], f32, kind="ExternalInput")
    bmask = nc.dram_tensor("bmask", [128, BAND], f32, kind="ExternalInput")
    out = nc.dram_tensor("out", [ROWS, D], f32, kind="ExternalOutput")

    def mm(ap):
        return ap.bitcast(mm_dt) if mm_dt is not f32 else ap

    with tile.TileContext(nc) as tc, ExitStack() as ctx:
        const = ctx.enter_context(tc.tile_pool(name="const", bufs=1))
        big = ctx.enter_context(tc.tile_pool(name="big", bufs=1))
        work = ctx.enter_context(tc.tile_pool(name="work", bufs=3))
        etp = ctx.enter_context(tc.tile_pool(name="etp", bufs=4))
        sb2 = ctx.enter_context(tc.tile_pool(name="sb2", bufs=2))
        ps_st = ctx.enter_context(tc.tile_pool(name="ps_st", bufs=3, space="PSUM"))
        ps_h = ctx.enter_context(tc.tile_pool(name="ps_h", bufs=1, space="PSUM"))
        ps_g = ctx.enter_context(tc.tile_pool(name="ps_g", bufs=1, space="PSUM"))
        ps_ws = ctx.enter_context(tc.tile_pool(name="ps_ws", bufs=2, space="PSUM"))

        # ---- persistent SBUF tensors ----
        zT_sb = big.tile([128, M], f32)
        zc_sb = big.tile([128, NQ, D], f32)
        zTh_sb = big.tile([128, HALO], f32)
        zoc_sb = big.tile([128, NL, D], f32)
        uT_sb = big.tile([128, ROWS], f32)
        wgT_sb = big.tile([128, HALO], f32)
        wgN_sb = big.tile([128, 10, D], f32)

        BT_sb = const.tile([128, D], f32)
        Wg_sb = const.tile([128, D], f32)
        Wc_sb = const.tile([128, D], f32)
        ac_sb = const.tile([128, 2], f32)
        bm_sb = const.tile([128, BAND], f32)
        id_sb = const.tile([128, 128], f32)
        ones_col = const.tile([128, 1], f32)
        ones_row = const.tile([1, 128], f32)
        ones_1 = const.tile([1, 1], f32)

        nc.sync.dma_start(zT_sb[:, :], zT[:, :])
        nc.sync.dma_start(zc_sb[:, :, :], zc[:, :, :])
        nc.sync.dma_start(zTh_sb[:, :], zTh[:, :])
        nc.sync.dma_start(zoc_sb[:, :, :], zoc[:, :, :])
        nc.sync.dma_start(BT_sb[:, :], BT[:, :])
        nc.sync.dma_start(Wg_sb[:, :], Wg_t[:, :])
        nc.sync.dma_start(Wc_sb[:, :], Wc_t[:, :])
        nc.sync.dma_start(ac_sb[:, :], a_cols[:, :])
        nc.sync.dma_start(bm_sb[:, :], bmask[:, :])
        make_identity(nc, id_sb[:, :])
        nc.vector.memset(ones_col[:, :], 1.0)
        nc.vector.memset(ones_row[:, :], 1.0)
        nc.vector.memset(ones_1[:, :], 1.0)

        # ---- preproc: uT = (Wq^T Wk)^T z_own^T ; wg^T halo ; wg halo rows ----
        for b in range(ROWS // 512):
            u_ps = ps_st.tile([128, J], f32, tag="st")
            nc.tensor.matmul(u_ps[:, :], mm(BT_sb[:, :]),
                             mm(zTh_sb[:, b * 512:(b + 1) * 512]),
                             start=True, stop=True)
            nc.any.tensor_copy(uT_sb[:, b * 512:(b + 1) * 512], u_ps[:, :])

        for off, w in ((0, 512), (512, 512), (1024, 256)):
            wg_ps = ps_st.tile([128, J], f32, tag="st")
            nc.tensor.matmul(wg_ps[:, :w], mm(Wg_sb[:, :]),
                             mm(zTh_sb[:, off:off + w]), start=True, stop=True)
            nc.any.tensor_copy(wgT_sb[:, off:off + w], wg_ps[:, :w])

        for q in range(9):
            wn_ps = ps_st.tile([128, J], f32, tag="st")
            nc.tensor.matmul(wn_ps[:, :D], mm(zTh_sb[:, 1 + 128 * q:129 + 128 * q]),
                             mm(Wg_sb[:, :]), start=True, stop=True)
            nc.any.tensor_copy(wgN_sb[:, q, :], wn_ps[:, :D])

        # ---- main loop over query blocks ----
        for jb in range(NB):
            js = jb * J
            h_ps = ps_h.tile([128, J], f32, tag="h")
            partial = sb2.tile([128, J], f32, tag="partial")
            for q in range(NQ):
                st_ps = ps_st.tile([128, J], f32, tag="st")
                nc.tensor.matmul(st_ps[:, :], mm(zT_sb[:, q * 128:(q + 1) * 128]),
                                 mm(uT_sb[:, js:js + J]), start=True, stop=True)
                et = etp.tile([128, J], f32, tag="et")
                nc.scalar.activation(et[:, :], st_ps[:, :], Act.Exp,
                                     bias=0.0, scale=SCALE)
                if q == 0:
                    nc.vector.tensor_copy(partial[:, :], et[:, :])
                else:
                    nc.vector.tensor_add(partial[:, :], partial[:, :], et[:, :])
                nc.tensor.matmul(h_ps[:, :], mm(zc_sb[:, q, :]), mm(et[:, :]),
                                 start=(q == 0), stop=(q == NQ - 1))

            h_sb = sb2.tile([128, J], f32, tag="h_sb")
            nc.vector.tensor_copy(h_sb[:, :], h_ps[:, :])
            gt_ps = ps_g.tile([128, J], f32, tag="gt")
            nc.tensor.matmul(gt_ps[:, :], mm(Wc_sb[:, :]), mm(h_sb[:, :]),
                             start=True, stop=True)
            gt_sb = sb2.tile([128, J], f32, tag="gt_sb")
            nc.any.tensor_copy(gt_sb[:, :], gt_ps[:, :])

            den_ps = ps_g.tile([1, J], f32, tag="den")
            nc.tensor.matmul(den_ps[:, :], mm(ones_col[:, :]), mm(partial[:, :]),
                             start=True, stop=True)
            rden = sb2.tile([1, J], f32, tag="rden")
            nc.vector.reciprocal(rden[:, :], den_ps[:, :])

            for t in range(NT):
                bi = jb * NT + t
                ws = ps_ws.tile([128, 512], f32, tag="ws")

                # --- banded local attention for rows [128*bi, 128*bi+128) ---
                s1_ps = ws[:, 448:449]
                nc.tensor.matmul(s1_ps, mm(wgT_sb[:, 128 * bi:128 * bi + 128]),
                                 mm(ac_sb[:, 0:1]), start=True, stop=True)
                s1_sb = work.tile([128, 1], f32, tag="s1")
                nc.any.tensor_copy(s1_sb[:, :], s1_ps)

                s2_ps = ws[0:1, 288:448]
                nc.tensor.matmul(s2_ps, mm(ac_sb[:, 1:2]),
                                 mm(wgT_sb[:, 128 * bi + 1:128 * bi + 1 + BAND]),
                                 start=True, stop=True)
                s2_sb = work.tile([1, BAND], f32, tag="s2")
                nc.any.tensor_copy(s2_sb[:, :], s2_ps)

                s2b_ps = ws[:, 0:BAND]
                nc.tensor.matmul(s2b_ps, mm(ones_row[:, :]), mm(s2_sb[:, :]),
                                 start=True, stop=True)
                band = work.tile([128, BAND], f32, tag="band")
                nc.scalar.activation(band[:, :], s2b_ps, Act.Identity,
                                     bias=s1_sb[:, :], scale=1.0)
                nc.vector.scalar_tensor_tensor(band[:, :], band[:, :], LEAK,
                                               band[:, :], Alu.mult, Alu.max)
                nc.vector.tensor_add(band[:, :], band[:, :], bm_sb[:, :])
                rmax = work.tile([128, 1], f32, tag="rmax")
                nc.vector.reduce_max(rmax[:, :], band[:, :], axis=AX)
                nmax = work.tile([128, 1], f32, tag="nmax")
                nc.vector.tensor_scalar_mul(nmax[:, :], rmax[:, :], -1.0)
                eb = work.tile([128, BAND], f32, tag="eb")
                nc.scalar.activation(eb[:, :], band[:, :], Act.Exp,
                                     bias=nmax[:, :], scale=1.0)
                dn = work.tile([128, 1], f32, tag="dn")
                nc.vector.reduce_sum(dn[:, :], eb[:, :], axis=AX)
                rd = work.tile([128, 1], f32, tag="rd")
                nc.vector.reciprocal(rd[:, :], dn[:, :])
                nc.vector.tensor_scalar_mul(eb[:, :], eb[:, :], rd[:, :])

                tr1 = ws[:, 160:288]
                nc.tensor.transpose(tr1, eb[:, 0:128], id_sb[:, :])
                tr2 = ws[0:32, 288:416]
                nc.tensor.transpose(tr2, eb[:, 128:BAND], id_sb[:, :])
                a1 = work.tile([128, 128], f32, tag="a1")
                nc.any.tensor_copy(a1[:, :], tr1)
                a2 = work.tile([32, 128], f32, tag="a2")
                nc.any.tensor_copy(a2[:, :], tr2)

                loc_ps = ws[:, 288:416]
                nc.tensor.matmul(loc_ps, mm(a1[:, :]), mm(wgN_sb[:, bi, :]),
                                 start=True, stop=False)
                nc.tensor.matmul(loc_ps, mm(a2[:, :]), mm(wgN_sb[0:32, bi + 1, :]),
                                 start=False, stop=True)
                loc_sb = work.tile([128, 128], f32, tag="loc")
                nc.any.tensor_copy(loc_sb[:, :], loc_ps)

                # --- final: leaky(gt^T/den + local + z) ---
                gtt_ps = ws[:, 160:288]
                nc.tensor.transpose(gtt_ps, gt_sb[:, t * 128:(t + 1) * 128],
                                    id_sb[:, :])
                rdt_ps = ws[:, 449:450]
                nc.tensor.matmul(rdt_ps, mm(rden[0:1, t * 128:(t + 1) * 128]),
                                 mm(ones_1[:, :]), start=True, stop=True)
                rdt_sb = work.tile([128, 1], f32, tag="rdt")
                nc.any.tensor_copy(rdt_sb[:, :], rdt_ps)

                fin = work.tile([128, 128], f32, tag="fin")
                nc.vector.scalar_tensor_tensor(fin[:, :], gtt_ps, rdt_sb[:, :],
                                               loc_sb[:, :], Alu.mult, Alu.add)
                nc.vector.tensor_add(fin[:, :], fin[:, :], zoc_sb[:, bi, :])
                nc.vector.scalar_tensor_tensor(fin[:, :], fin[:, :], LEAK,
                                               fin[:, :], Alu.mult, Alu.max)
                nc.sync.dma_start(out[128 * bi:128 * (bi + 1), :], fin[:, :])

    nc.compile()
    return nc


def _get_nc():
    if "nc" not in _CACHE:
        _CACHE["nc"] = _build_nc()
    return _CACHE["nc"]


def _make_in_maps(z, Wg, Wc, Wq, Wk, a):
    z = np.ascontiguousarray(np.asarray(z, dtype=np.float32))
    Wg = np.asarray(Wg, dtype=np.float32)
    Wc = np.asarray(Wc, dtype=np.float32)
    Wq = np.asarray(Wq, dtype=np.float32)
    Wk = np.asarray(Wk, dtype=np.float32)
    a = np.asarray(a, dtype=np.float32)

    zT = np.ascontiguousarray(z.T)                                   # [D, M]
    zc = np.ascontiguousarray(z.reshape(NQ, 128, D).transpose(1, 0, 2))
    BT = (Wq.astype(np.float64).T @ Wk.astype(np.float64)).astype(np.float32)
    Wg_t = np.ascontiguousarray(Wg.T)
    Wc_t = np.ascontiguousarray(Wc.T)
    a_cols = np.ascontiguousarray(np.stack([a[:D], a[D:]], axis=1))

    jj = np.arange(128)[:, None]
    cc = np.arange(BAND)[None, :]
    bmask = np.where((cc >= jj) & (cc <= jj + DEG - 1), 0.0, -1e30)
    bmask = bmask.astype(np.float32)

    shared = dict(zT=zT, zc=zc, BT=BT, Wg_t=Wg_t, Wc_t=Wc_t,
                  a_cols=a_cols, bmask=bmask)
    in_maps = []
    for core in range(NCORES):
        r0 = core * ROWS
        idx = (r0 + np.arange(HALO)) % M
        zTh = np.ascontiguousarray(zT[:, idx])
        zoc = np.ascontiguousarray(
            z[r0:r0 + ROWS].reshape(NL, 128, D).transpose(1, 0, 2))
        in_maps.append(dict(shared, zTh=zTh, zoc=zoc))
    return in_maps


def _run(z, Wg, Wc, Wq, Wk, a, trace=False, **kwargs):
    from concourse.bass_utils import run_bass_kernel_spmd
    nc = _get_nc()
    in_maps = _make_in_maps(z, Wg, Wc, Wq, Wk, a)
    res = run_bass_kernel_spmd(nc, in_maps, core_ids=list(range(NCORES)),
                               trace=trace, **kwargs)
    outp = np.concatenate([res.results[i]["out"] for i in range(NCORES)], axis=0)
    return outp.astype(np.float32), res


def _expected_edges(edge_index):
    ei = np.asarray(edge_index).astype(np.int64)
    if ei.shape != (2, M * DEG):
        return False
    src = np.repeat(np.arange(M, dtype=np.int64), DEG)
    dst = (src + np.tile(np.arange(1, DEG + 1, dtype=np.int64), M)) % M
    return bool(np.array_equal(ei[0], src) and np.array_equal(ei[1], dst))


def _leaky(x):
    return np.where(x > 0, x, LEAK * x)


def _numpy_fallback(z, edge_index, Wg, Wc, Wq, Wk, a):
    # General-edge fallback (not expected to trigger with the shipped inputs).
    z = np.asarray(z, dtype=np.float32)
    ei = np.asarray(edge_index).astype(np.int64)
    Wg = np.asarray(Wg, np.float32); Wc = np.asarray(Wc, np.float32)
    Wq = np.asarray(Wq, np.float32); Wk = np.asarray(Wk, np.float32)
    a = np.asarray(a, np.float32)
    m, d = z.shape
    wg = z @ Wg.T
    src, dst = ei[0], ei[1]
    scores = _leaky((wg @ a[:d])[src] + (wg @ a[d:])[dst])
    smax = np.full(m, -np.inf, np.float32)
    np.maximum.at(smax, src, scores)
    ex = np.exp(scores - smax[src])
    denom = np.zeros(m, np.float32)
    np.add.at(denom, src, ex)
    alpha = ex / denom[src]
    local = np.zeros((m, d), np.float32)
    np.add.at(local, src, alpha[:, None] * wg[dst])
    q = z @ Wq.T
    k = z @ Wk.T
    s = (q @ k.T) / np.sqrt(np.float32(d))
    s = s - s.max(axis=-1, keepdims=True)
    e = np.exp(s)
    beta = e / e.sum(axis=-1, keepdims=True)
    gmsg = beta @ (z @ Wc.T)
    return _leaky(local + gmsg + z).astype(np.float32)


def kernel(z, edge_index, Wg, Wc, Wq, Wk, a):
    if not _expected_edges(edge_index):
        return _numpy_fallback(z, edge_index, Wg, Wc, Wq, Wk, a)
    outp, _ = _run(z, Wg, Wc, Wq, Wk, a, trace=False)
    return outp
